# revision 10
# baseline (speedup 1.0000x reference)
"""GAT message-passing kernel for 8 Trainium2 NeuronCores.

Problem (nn_GAT_PointGeo): N=10000 nodes, E=160000 edges, D=512.
  x_src = x @ W_src + b_src ; x_dst = x @ W_dst + b_dst
  alpha_e = softmax_over_dst( x_src[src_e] . x_dst[dst_e] / sqrt(D) )
  z_i     = sum_{e: dst_e=i} alpha_e * x_src[src_e]
  pred    = (z @ W_pred + b_pred) * (tg_mask == 1)

Sharding: edges partitioned by destination node (1250 dst/core).  Each core
computes the full bias-free projected source table xs = x @ W_src twice to
DRAM (fp16 row-major for aggregation gathers; fp8e4m3 for score gathers),
plus the transposed local destination projection xdT = (W_dst/temp)^T x^T
+ bd (SBUF-resident; W_dst columns host-permuted into the fp8 16-bit-pair
order).  Edge phase per 128-dst tile (KT k-tiles of 128 slots):
  - hj  = xs[src]   fp16 via SWDGE dma_gather (queue 0), [slot, D]
  - hjT = xs8[src]^T fp8 via SWDGE dma_gather(transpose=True, queue 1)
  - S[dst, slot] = xdT^T @ hjT on the tensor engine (4 matmuls), plus a
    host-built additive mask B (one-hot -> -SHIFT, else -60) accumulated
    into PSUM via an identity-weight matmul
  - V = Exp(S + cor) on the scalar engine (cor = per-dst b_src score
    correction, precomputed per tile), denominator via accum_out
  - V^T via PE transposes -> lhsT of the weighted-aggregation matmuls
    z[dst, D] = V^T.T @ hj (17 matmuls)
  - prediction head on vector engine; b_src aggregation bias folded in as
    dtot * (b_src @ W_pred) rank-1 correction.
"""

import math
import sys

import numpy as np

sys.path.insert(0, "/opt/trn_rl_repo")

N, E, D = 10000, 160000, 512
NCORES = 8
P = 128
KD = D // P                 # 4 contraction chunks of 128
NL = N // NCORES            # 1250 local dst nodes / core
DT = (NL + P - 1) // P      # 10 dst tiles / core
NLP = DT * P                # 1280 padded local rows
NCH = 10                    # projection node chunks of 1024
NPAD = NCH * 1024           # 10240 padded source-table rows
TEMP = math.sqrt(float(D))
SHIFT = 4.0                 # global logit shift (softmax invariant)
NEG = -60.0                 # mask for non-matching / padded slots

_NC_CACHE = {}


def build_nc(KT):
    """Build the (SPMD, per-core-uniform) Bass program.  KT = k-tiles of 128
    edge slots per dst tile (compile-time, data-dependent)."""
    import concourse.bacc as bacc
    import concourse.mybir as mybir
    from concourse import tile
    from contextlib import ExitStack

    fp16 = mybir.dt.float16
    fp8 = mybir.dt.float8e4
    f32 = mybir.dt.float32
    i16 = mybir.dt.int16
    Alu = mybir.AluOpType
    Act = mybir.ActivationFunctionType

    nc = bacc.Bacc("TRN2", dynamic_dma_scratch_size=65536, num_swdge_queues=2)

    NIDX = KT * P
    NCHK = (KT + 3) // 4        # score chunks of <=4 k-tiles (psum bank)
    TSET = {2, 5, 8}            # tiles whose hjT comes from PE transposes

    # ---- I/O ----------------------------------------------------------
    xT = nc.dram_tensor("xT", [P, NCH * KD * 1024], fp16, kind="ExternalInput")
    ws = nc.dram_tensor("ws", [P, KD * 512], fp16, kind="ExternalInput")
    wdT = nc.dram_tensor("wdT", [P, KD * KD * P], fp16, kind="ExternalInput")
    bdT = nc.dram_tensor("bdT", [P, KD], f32, kind="ExternalInput")
    bsT = nc.dram_tensor("bsT", [P, KD], fp16, kind="ExternalInput")
    srcidx = nc.dram_tensor("srcidx", [P, DT * KT * 8], i16, kind="ExternalInput")
    Bm = nc.dram_tensor("Bm", [P, DT * KT * P], fp16, kind="ExternalInput")
    ident = nc.dram_tensor("ident", [P, P], fp16, kind="ExternalInput")
    wp = nc.dram_tensor("wp", [P, 2 * D], f32, kind="ExternalInput")
    bp = nc.dram_tensor("bp", [P, 2], f32, kind="ExternalInput")
    bwp = nc.dram_tensor("bwp", [P, 2], f32, kind="ExternalInput")
    tg = nc.dram_tensor("tg", [P, DT], f32, kind="ExternalInput")
    pred_out = nc.dram_tensor("pred_out", [DT, P, 2], f32, kind="ExternalOutput")

    xs_dram = nc.dram_tensor("xs_dram", [NPAD, D], fp16, kind="Internal")

    with tile.TileContext(nc) as tc, ExitStack() as ctx:
        pool = lambda name, bufs, **kw: ctx.enter_context(
            tc.tile_pool(name=name, bufs=bufs, **kw)
        )
        const = pool("const", 1)

        # constants / small inputs -> SBUF
        ws_s = const.tile([P, KD * 512], fp16)
        nc.sync.dma_start(ws_s[:], ws[:])
        wdT_s = const.tile([P, KD * KD * P], fp16)
        nc.sync.dma_start(wdT_s[:], wdT[:])
        bdT_s = const.tile([P, KD], f32)
        nc.sync.dma_start(bdT_s[:], bdT[:])
        bsT_s = const.tile([P, KD], fp16)
        nc.sync.dma_start(bsT_s[:], bsT[:])
        srcidx_s = const.tile([P, DT * KT * 8], i16)
        nc.sync.dma_start(srcidx_s[:], srcidx[:])
        ident_s = const.tile([P, P], fp16)
        nc.sync.dma_start(ident_s[:], ident[:])
        wp_s = const.tile([P, 2 * D], f32)
        nc.sync.dma_start(wp_s[:], wp[:])
        bp_s = const.tile([P, 2], f32)
        nc.sync.dma_start(bp_s[:], bp[:])
        bwp_s = const.tile([P, 2], f32)
        nc.sync.dma_start(bwp_s[:], bwp[:])
        tg_s = const.tile([P, DT], f32)
        nc.sync.dma_start(tg_s[:], tg[:])

        xdT_s = const.tile([P, KD, NLP], fp16)   # SBUF-resident xdT
        corAll_s = const.tile([P, DT], f32)      # per-tile b_src score corr.

        with ExitStack() as p1ctx:
            p1pool = lambda name, bufs, **kw: p1ctx.enter_context(
                tc.tile_pool(name=name, bufs=bufs, **kw)
            )
            xtpin_pool = p1pool("xtpin", 2)
            xt_pool = p1pool("xt", 3)
            pps_pool = p1pool("pps", 3, space="PSUM")
            dps_pool = p1pool("dps", 2, space="PSUM")
            cor_pool = p1pool("corps", 1, space="PSUM")
            stage_pool = p1pool("stage", 2)

            # ---- Phase 1a: xs tables (bias-free), fp16 + fp8 ----------
            xt0 = xtpin_pool.tile([P, KD * 1024], fp16, tag="xtpin")
            nc.sync.dma_start(xt0[:], xT[:, 0:KD * 1024])
            xt1 = xtpin_pool.tile([P, KD * 1024], fp16, tag="xtpin")
            nc.sync.dma_start(xt1[:], xT[:, KD * 1024:2 * KD * 1024])
            for ch in range(NCH):
                if ch == 0:
                    xt_s = xt0
                elif ch == 1:
                    xt_s = xt1
                else:
                    xt_s = xt_pool.tile([P, KD * 1024], fp16, tag="xt")
                    nc.sync.dma_start(
                        xt_s[:], xT[:, ch * KD * 1024:(ch + 1) * KD * 1024]
                    )
                stage = stage_pool.tile([P, 8, D], fp16)
                for m in range(8):
                    ps = pps_pool.tile([P, 512], f32, tag="pps")
                    for k in range(KD):
                        nc.tensor.matmul(
                            ps[:],
                            xt_s[:, k * 1024 + m * P: k * 1024 + (m + 1) * P],
                            ws_s[:, k * 512:(k + 1) * 512],
                            start=(k == 0), stop=(k == KD - 1),
                        )
                    nc.scalar.activation(stage[:, m, :], ps[:], Act.Copy)
                nc.sync.dma_start(
                    xs_dram[ch * 1024:(ch + 1) * 1024, :].rearrange(
                        "(m p) f -> p m f", p=P
                    ),
                    stage[:],
                )

            # ---- Phase 1b: xdT = (W_dst/temp)^T @ x^T + bd ------------
            # (feature order = fp8 pair permutation, baked into wdT/bdT)
            blocks = [(0, 0, 0, 512), (512, 0, 512, 512), (1024, 1, 0, 256)]
            for q in range(KD):
                for n0, chb, j0, w in blocks:
                    xtb = xt0 if chb == 0 else xt1
                    ps = dps_pool.tile([P, 512], f32, tag="dps")
                    for k in range(KD):
                        nc.tensor.matmul(
                            ps[:, 0:w],
                            wdT_s[:, (k * KD + q) * P:(k * KD + q + 1) * P],
                            xtb[:, k * 1024 + j0: k * 1024 + j0 + w],
                            start=(k == 0), stop=(k == KD - 1),
                        )
                    nc.scalar.activation(
                        xdT_s[:, q, n0:n0 + w], ps[:, 0:w], Act.Identity,
                        bias=bdT_s[:, q:q + 1],
                    )

            # ---- Phase 1c: per-tile score corrections cor = xdT^T bs --
            corps = cor_pool.tile([P, DT], f32)
            for t in range(DT):
                for q in range(KD):
                    nc.tensor.matmul(
                        corps[:, t:t + 1],
                        xdT_s[:, q, t * P:(t + 1) * P],
                        bsT_s[:, q:q + 1],
                        start=(q == 0), stop=(q == KD - 1),
                    )
            nc.scalar.activation(corAll_s[:], corps[:], Act.Copy)

        # ---- Phase 2: edge phase per dst tile ------------------------
        hj_pool = pool("hj", 3)
        hjt_pool = pool("hjt", 3)
        bm_pool = pool("bm", 3)
        v_pool = pool("v", 2)
        vt_pool = pool("vt", 2)
        sps_pool = pool("sps", 2, space="PSUM")
        htp_pool = pool("htp", 1, space="PSUM")
        vtp_pool = pool("vtp", 2, space="PSUM")
        zps_pool = pool("zps", 2, space="PSUM")
        small_pool = pool("small", 2)
        junk_pool = pool("junk", 2)
        out_pool = pool("out", 2)

        for t in range(DT):
            hj = hj_pool.tile([P, KT, D], fp16)
            nc.gpsimd.dma_gather(
                hj[:], xs_dram[:], srcidx_s[:, t * KT * 8:(t + 1) * KT * 8],
                NIDX, NIDX, D, single_packet=False, queue_num=0,
            )
            hjT = hjt_pool.tile([P, KD, NIDX], fp16)
            if t in TSET:
                # derive hjT on-chip: PE-transpose hj 128x128 blocks
                for g in range((KT + 3) // 4):
                    gk = min(4, KT - g * 4)
                    htp = htp_pool.tile([P, KD, 4, P], fp16, tag="htp")
                    for jj in range(gk):
                        j = g * 4 + jj
                        for k in range(KD):
                            nc.tensor.transpose(
                                htp[:, k, jj, :],
                                hj[:, j, k * P:(k + 1) * P], ident_s[:],
                            )
                    if g % 2 == 0:
                        nc.scalar.activation(
                            hjT[:, :, g * 4 * P:g * 4 * P + gk * P],
                            htp[:, :, 0:gk, :], Act.Copy,
                        )
                    else:
                        nc.vector.tensor_copy(
                            hjT[:, :, g * 4 * P:g * 4 * P + gk * P],
                            htp[:, :, 0:gk, :],
                        )
            else:
                nc.gpsimd.dma_gather(
                    hjT[:], xs_dram[:],
                    srcidx_s[:, t * KT * 8:(t + 1) * KT * 8],
                    NIDX, NIDX, D, transpose=True, single_packet=False,
                    queue_num=1,
                )
            bm_t = bm_pool.tile([P, KT * P], fp16)
            nc.sync.dma_start(bm_t[:], Bm[:, t * KT * P:(t + 1) * KT * P])

            vt_sb = vt_pool.tile([P, KT, P], fp16)
            dsums = small_pool.tile([P, NCHK], f32, tag="dsums")
            for ci in range(NCHK):
                kk = min(4, KT - ci * 4)
                cw = kk * P
                s0 = ci * 4 * P
                sps = sps_pool.tile([P, 512], f32, tag="sps")
                for q in range(KD):
                    nc.tensor.matmul(
                        sps[:, 0:cw],
                        xdT_s[:, q, t * P:(t + 1) * P],
                        hjT[:, q, s0:s0 + cw],
                        start=(q == 0), stop=False,
                    )
                nc.tensor.matmul(
                    sps[:, 0:cw], ident_s[:], bm_t[:, s0:s0 + cw],
                    start=False, stop=True,
                )
                vch = v_pool.tile([P, 512], fp16, tag="vch")
                nc.scalar.activation(
                    vch[:, 0:cw], sps[:, 0:cw], Act.Exp,
                    bias=corAll_s[:, t:t + 1], accum_out=dsums[:, ci:ci + 1],
                )
                vtp = vtp_pool.tile([P, 4, P], fp16, tag="vtp")
                for j in range(kk):
                    nc.tensor.transpose(
                        vtp[:, j, :], vch[:, j * P:(j + 1) * P], ident_s[:]
                    )
                nc.scalar.activation(
                    vt_sb[:, ci * 4:ci * 4 + kk, :], vtp[:, 0:kk, :], Act.Copy
                )

            zps = zps_pool.tile([P, D], f32)
            for j in range(KT):
                nc.tensor.matmul(
                    zps[:], vt_sb[:, j, :], hj[:, j, :],
                    start=(j == 0), stop=(j == KT - 1),
                )

            # denom = sum of chunk sums, reciprocal
            junk5 = small_pool.tile([P, NCHK], f32, tag="junk5")
            dtot = small_pool.tile([P, 1], f32, tag="dtot")
            nc.scalar.activation(
                junk5[:], dsums[:], Act.Copy, accum_out=dtot[:]
            )
            dr = small_pool.tile([P, 1], f32, tag="dr")
            nc.vector.tensor_scalar_add(dr[:], dtot[:], 1e-16)
            dr2 = small_pool.tile([P, 1], f32, tag="dr2")
            nc.vector.reciprocal(dr2[:], dr[:])

            # head: pred = ((z + dtot*bs) @ W_pred) * dr * tg + b_pred * tg
            junkD = junk_pool.tile([P, D], f32, tag="junkD")
            praw = small_pool.tile([P, 2], f32, tag="praw")
            s1 = small_pool.tile([P, 2], f32, tag="s1")
            t1 = small_pool.tile([P, 2], f32, tag="t1")
            pred_sb = out_pool.tile([P, 2], f32)
            for c in range(2):
                nc.vector.scalar_tensor_tensor(
                    out=junkD[:], in0=zps[:], scalar=1.0,
                    in1=wp_s[:, c * D:(c + 1) * D],
                    op0=Alu.mult, op1=Alu.mult,
                    accum_out=praw[:, c:c + 1],
                )
                nc.vector.scalar_tensor_tensor(
                    out=s1[:, c:c + 1], in0=dtot[:], scalar=bwp_s[:, c:c + 1],
                    in1=praw[:, c:c + 1], op0=Alu.mult, op1=Alu.add,
                )
                nc.vector.scalar_tensor_tensor(
                    out=t1[:, c:c + 1], in0=s1[:, c:c + 1], scalar=dr2[:],
                    in1=tg_s[:, t:t + 1], op0=Alu.mult, op1=Alu.mult,
                )
                nc.vector.scalar_tensor_tensor(
                    out=pred_sb[:, c:c + 1], in0=tg_s[:, t:t + 1],
                    scalar=bp_s[:, c:c + 1], in1=t1[:, c:c + 1],
                    op0=Alu.mult, op1=Alu.add,
                )
            nc.sync.dma_start(pred_out[t, :, :], pred_sb[:])

    nc.compile()
    return nc


def prep_inputs(x, edge_index, tg_mask, W_src, b_src, W_dst, b_dst, W_pred, b_pred):
    """Host-side sharding/layout prep.  Returns (KT, in_maps)."""
    x = np.asarray(x, np.float32)
    src = np.asarray(edge_index[0], np.int64)
    dst = np.asarray(edge_index[1], np.int64)
    tgm = (np.asarray(tg_mask) == 1).astype(np.float32)
    W_src = np.asarray(W_src, np.float32)
    W_dst = np.asarray(W_dst, np.float32)
    b_src = np.asarray(b_src, np.float32)
    b_dst = np.asarray(b_dst, np.float32)
    W_pred = np.asarray(W_pred, np.float32)
    b_pred = np.asarray(b_pred, np.float32)

    order = np.argsort(dst, kind="stable")
    src_s, dst_s = src[order], dst[order]

    # per-(core, tile) edge lists -> global KT
    cores = []
    KT = 1
    for c in range(NCORES):
        lo, hi = c * NL, (c + 1) * NL
        sel = (dst_s >= lo) & (dst_s < hi)
        cs, cd = src_s[sel], dst_s[sel] - lo
        tiles = []
        for t in range(DT):
            m = (cd >= t * P) & (cd < (t + 1) * P)
            tcs, tcd = cs[m], cd[m] - t * P
            # dedup by (src, dst); slot = unique src, k = multiplicity
            usrc, sinv = np.unique(tcs, return_inverse=True)
            pair = sinv.astype(np.int64) * P + tcd
            upair, pcnt = np.unique(pair, return_counts=True)
            tiles.append((usrc, upair // P, upair % P, pcnt))
            KT = max(KT, (usrc.size + P - 1) // P)
        cores.append(tiles)

    # shared weight layouts (W_dst/b_src/b_dst in fp8 pair-feature order)
    ws_np = np.ascontiguousarray(
        W_src.reshape(KD, P, 512).transpose(1, 0, 2)
    ).astype(np.float16).reshape(P, KD * 512)
    wdT_np = np.ascontiguousarray(
        (W_dst / TEMP).reshape(KD, P, KD, P).transpose(1, 0, 2, 3)
    ).astype(np.float16).reshape(P, KD * KD * P)
    bdT_np = np.ascontiguousarray(
        (b_dst / TEMP).reshape(KD, P).T
    ).astype(np.float32)
    bsT_np = np.ascontiguousarray(b_src.reshape(KD, P).T).astype(np.float16)
    ident_np = np.eye(P, dtype=np.float16)
    wp_np = np.broadcast_to(
        W_pred.T.reshape(1, 2 * D), (P, 2 * D)
    ).astype(np.float32).copy()
    bp_np = np.broadcast_to(b_pred[None, :], (P, 2)).astype(np.float32).copy()
    bwp_np = np.broadcast_to(
        (b_src @ W_pred)[None, :], (P, 2)
    ).astype(np.float32).copy()

    in_maps = []
    for c in range(NCORES):
        lo = c * NL
        perm = np.concatenate(
            [np.arange(lo, lo + NL), np.arange(0, lo), np.arange(lo + NL, N)]
        )
        pos = np.empty(N, np.int64)
        pos[perm[:NL]] = np.arange(NL)
        pos[perm[NL:]] = NLP + np.arange(N - NL)

        x_perm = np.zeros((NPAD, D), np.float32)
        x_perm[:NL] = x[perm[:NL]]
        x_perm[NLP: NLP + (N - NL)] = x[perm[NL:]]
        # xT layout: [p, ch*KD*1024 + k*1024 + j] = x_perm[ch*1024+j, k*128+p]
        xt_np = np.ascontiguousarray(
            x_perm.reshape(NCH, 1024, KD, P).transpose(3, 0, 2, 1)
        ).astype(np.float16).reshape(P, NCH * KD * 1024)

        sidx = np.zeros((DT, KT * P), np.int16)
        bmask = np.full((DT, P, KT * P), NEG, np.float16)
        for t in range(DT):
            usrc, pslot, pdst, pcnt = cores[c][t]
            n = usrc.size
            sidx[t, :n] = pos[usrc]
            bmask[t, pdst, pslot] = (-SHIFT + np.log(pcnt)).astype(np.float16)

        def wrap(a):  # [DT, KT*P] -> [P, DT*KT*8] int16 wrapped/replicated
            w = np.ascontiguousarray(
                a.reshape(DT, KT * 8, 16).transpose(0, 2, 1)
            )  # [DT, 16, KT*8]
            w = np.tile(w[:, None, :, :], (1, 8, 1, 1)).reshape(DT, P, KT * 8)
            return np.ascontiguousarray(w.transpose(1, 0, 2)).reshape(P, DT * KT * 8)

        bm_np = np.ascontiguousarray(
            bmask.transpose(1, 0, 2)
        ).reshape(P, DT * KT * P)

        tg_np = np.zeros((P, DT), np.float32)
        tgl = tgm[lo: lo + NL]
        full = np.zeros(NLP, np.float32)
        full[:NL] = tgl
        tg_np[:] = full.reshape(DT, P).T

        in_maps.append(dict(
            xT=xt_np, ws=ws_np, wdT=wdT_np, bdT=bdT_np, bsT=bsT_np,
            srcidx=wrap(sidx), Bm=bm_np, ident=ident_np,
            wp=wp_np, bp=bp_np, bwp=bwp_np, tg=tg_np,
        ))
    return KT, in_maps


def assemble(results):
    out = np.zeros((N, 2), np.float32)
    for c in range(NCORES):
        blk = np.asarray(results[c]["pred_out"], np.float32).reshape(NLP, 2)
        out[c * NL:(c + 1) * NL] = blk[:NL]
    return out


def kernel(x, edge_index, tg_mask, W_src, b_src, W_dst, b_dst, W_pred, b_pred,
           trace=False):
    from concourse.bass_utils import run_bass_kernel_spmd

    KT, in_maps = prep_inputs(
        x, edge_index, tg_mask, W_src, b_src, W_dst, b_dst, W_pred, b_pred
    )
    if KT not in _NC_CACHE:
        _NC_CACHE[KT] = build_nc(KT)
    nc = _NC_CACHE[KT]
    res = run_bass_kernel_spmd(
        nc, in_maps, core_ids=list(range(NCORES)), trace=trace
    )
    kernel.last_result = res
    return assemble(res.results)
